# revision 9
# baseline (speedup 1.0000x reference)
"""Cluster-local attention kernel for Trainium2 (8 NeuronCores, SPMD).

Strategy
--------
Host side (numpy, cheap): replicate the reference's static window layout
(argsort by cluster label, bincount, big-cluster splitting), pack the ragged
windows into 128-slot tiles (best-fit decreasing), and split tiles evenly
across the 8 cores.  Attention is strictly window-local, so each tile is an
independent block-diagonal attention problem described by per-slot arrays:
source row in x (gather index), window id within the tile (mask), and
validity (for host-side compaction of the output).

Device side (Bass/Tile, uniform SPMD program, per-core data):
  per 4-tile group (512 token slots):
    - indirect-DMA gather of x rows -> X [128,384] f32 (kept for residual)
    - cast to bf16, PE-transpose -> xT [128, 3, 512] channel-major
    - QKV projection (bf16 matmuls, fp32 PSUM):
        q,k channel-major [128ch, 8 chunks, 512tok] (heads padded 96->128,
        1/sqrt(D) folded into wq host-side); v token-major [128tok, 384]
    - per tile: scoresT = k_h q_h^T (transposed scores, j on partitions),
      exp on ScalarE (no max subtraction needed -- scores are O(1)),
      block-diagonal mask multiply (window-id equality; the id row is
      materialized by a PE transpose of the id column), denominator via an
      all-ones [128,96] matmul (materializes the den row broadcast across
      96 PSUM partitions for free), reciprocal on DVE, out_hT = v_h^T
      probsT with the 1/den multiply fused into the PSUM->SBUF copy,
      out-projection, residual add, contiguous store to a staging buffer.
Host compacts the staging buffers (drops pad slots) and scatters rows to
their cluster-sorted positions (the reference returns cluster-sorted order).

The Pool engine is used ONLY to issue indirect-gather DMAs -- this image
ships no GpSimd HIPI ucode, so no Pool compute instructions are used.
"""

import os
import sys

sys.path.insert(0, "/opt/trn_rl_repo")

import numpy as np
import ml_dtypes

import concourse.bass as bass
import concourse.tile as tile
from concourse import bacc
from concourse import mybir
from concourse.bass import IndirectOffsetOnAxis
from concourse.bass_utils import run_bass_kernel_spmd

# problem constants (hardcoded per harness contract)
B, L, C, H, NCLUST, CS = 1, 32768, 384, 4, 512, 64
D = C // H  # 96
P = 128
NCORES = 8

f32 = mybir.dt.float32
bf16 = mybir.dt.bfloat16
i32 = mybir.dt.int32

_last_results = None  # stash of BassKernelResults for test.py introspection


# ----------------------------------------------------------------------------
# host-side window layout (replicates reference._windows)
# ----------------------------------------------------------------------------

def _window_layout(labels):
    """Return (index, starts, sizes) of the ragged windows over sorted order."""
    index = np.argsort(labels, kind="stable")
    sizes = np.bincount(labels).tolist()
    new = []
    for s in sizes:
        if s >= CS * 2:
            n = s // CS
            q, r = divmod(s, n)
            new.extend([q + 1 if i < r else q for i in range(n)])
        elif s > 0:
            new.append(s)
    sizes = np.asarray(new, np.int64)
    starts = np.concatenate([[0], np.cumsum(sizes)[:-1]])
    return index, starts, sizes


def _pack_bins(sizes, cap=P):
    """Best-fit decreasing packing of windows into bins of `cap` slots."""
    order = np.argsort(-sizes, kind="stable")
    rem = []
    bins = []
    for w in order:
        s = int(sizes[w])
        best = -1
        best_rem = cap + 1
        for bi, r in enumerate(rem):
            if s <= r < best_rem:
                best, best_rem = bi, r
        if best < 0:
            bins.append([int(w)])
            rem.append(cap - s)
        else:
            bins[best].append(int(w))
            rem[best] -= s
    return bins


def _build_core_data(labels):
    """Build per-core slot arrays (see module docstring)."""
    index, starts, sizes = _window_layout(labels)
    bins = _pack_bins(sizes)
    core_bins = [bins[c::NCORES] for c in range(NCORES)]
    T = max(len(cb) for cb in core_bins)
    T = ((T + 3) // 4) * 4  # kernel processes 4-tile groups

    per_core = []
    for c in range(NCORES):
        srci = np.zeros((T, P), np.int32)
        wid = np.full((T, P), -1.0, np.float32)
        valid = np.zeros((T, P), bool)
        gpos = []
        for t, b in enumerate(core_bins[c]):
            off = 0
            for k, w in enumerate(b):
                s = int(sizes[w])
                sl = slice(off, off + s)
                srci[t, sl] = index[starts[w]: starts[w] + s]
                wid[t, sl] = float(k)
                valid[t, sl] = True
                gpos.extend(range(int(starts[w]), int(starts[w]) + s))
                off += s
        per_core.append(dict(
            srci=np.ascontiguousarray(srci.T),          # [P, T]
            widc=np.ascontiguousarray(wid.T),           # [P, T]
            gpos=np.asarray(gpos, np.int64),
            valid=valid.reshape(-1),
        ))
    return T, per_core


# ----------------------------------------------------------------------------
# weight reorganization (host side)
# ----------------------------------------------------------------------------

def _prep_weights(w_qkv, w_out):
    """Reorganize weights into the SBUF layouts the kernel consumes (bf16)."""
    wq = w_qkv[:, :C] / np.sqrt(np.float32(D))
    wk = w_qkv[:, C:2 * C]
    wv = w_qkv[:, 2 * C:]

    qk = np.zeros((C, 8, P), np.float32)
    for h in range(H):
        qk[:, h, :D] = wq[:, h * D:(h + 1) * D]
        qk[:, 4 + h, :D] = wk[:, h * D:(h + 1) * D]
    qk = qk.reshape(C, 8 * P)                         # [384, 1024]
    wqk = qk.reshape(3, P, 8 * P).transpose(1, 0, 2)  # [128, 3, 1024]

    wv_r = wv.reshape(3, P, C).transpose(1, 0, 2)     # [128, 3, 384]
    wo_r = w_out.reshape(H, D, C).transpose(1, 0, 2)  # [96, 4, 384]

    bf = ml_dtypes.bfloat16
    return (np.ascontiguousarray(wqk).astype(bf),
            np.ascontiguousarray(wv_r).astype(bf),
            np.ascontiguousarray(wo_r).astype(bf))


# ----------------------------------------------------------------------------
# Bass program
# ----------------------------------------------------------------------------

def build_program(T):
    """Build the uniform SPMD Bass program for T tiles (T % 4 == 0)."""
    assert T % 4 == 0
    G = T // 4
    nc = bacc.Bacc("TRN2", target_bir_lowering=False)

    x = nc.dram_tensor("x", [L, C], f32, kind="ExternalInput")
    wqk = nc.dram_tensor("wqk", [P, 3, 8 * P], bf16, kind="ExternalInput")
    wv = nc.dram_tensor("wv", [P, 3, C], bf16, kind="ExternalInput")
    wo = nc.dram_tensor("wo", [D, H, C], bf16, kind="ExternalInput")
    srci = nc.dram_tensor("srci", [P, T], i32, kind="ExternalInput")
    widc = nc.dram_tensor("widc", [P, T], f32, kind="ExternalInput")
    identb = nc.dram_tensor("identb", [P, P], bf16, kind="ExternalInput")
    identf = nc.dram_tensor("identf", [P, P], f32, kind="ExternalInput")
    ones96 = nc.dram_tensor("ones96", [P, D], bf16, kind="ExternalInput")
    y_out = nc.dram_tensor("y", [T * P, C], f32, kind="ExternalOutput")

    ExpF = mybir.ActivationFunctionType.Exp

    with tile.TileContext(nc) as tc:
        with (
            tc.tile_pool(name="const", bufs=1) as cpool,
            tc.tile_pool(name="sb", bufs=2) as pool,
            tc.tile_pool(name="ps", bufs=1, space="PSUM") as psum,
        ):
            # ---- constants ----
            wqk_sb = cpool.tile([P, 3, 8 * P], bf16)
            nc.sync.dma_start(wqk_sb[:], wqk[:])
            wv_sb = cpool.tile([P, 3, C], bf16)
            nc.sync.dma_start(wv_sb[:], wv[:])
            wo_sb = cpool.tile([D, H, C], bf16)
            nc.sync.dma_start(wo_sb[:], wo[:])
            srci_sb = cpool.tile([P, T], i32)
            nc.sync.dma_start(srci_sb[:], srci[:])
            widc_sb = cpool.tile([P, T], f32)
            nc.sync.dma_start(widc_sb[:], widc[:])
            identb_sb = cpool.tile([P, P], bf16)
            nc.sync.dma_start(identb_sb[:], identb[:])
            identf_sb = cpool.tile([P, P], f32)
            nc.sync.dma_start(identf_sb[:], identf[:])
            ones96_sb = cpool.tile([P, D], bf16)
            nc.sync.dma_start(ones96_sb[:], ones96[:])

            # ---- PE warmups ----
            # TRN2 Matmult carries at most one sync-wait command, so make the
            # PE observe each constant's DMA semaphore via a single-input
            # dummy op before any real matmul consumes it.
            warm = psum.tile([P, P], f32, tag="small", space="PSUM")
            nc.tensor.transpose(warm[:], identf_sb[:], identf_sb[:])
            warmb = psum.tile([P, P], bf16, tag="small", space="PSUM")
            nc.tensor.transpose(warmb[:], identb_sb[:], identb_sb[:])
            w1 = psum.tile([P, P], f32, tag="small", space="PSUM")
            nc.tensor.matmul(w1[:], lhsT=wqk_sb[:, 0, 0:P], rhs=wqk_sb[:, 0, 0:P],
                             start=True, stop=True)
            w2 = psum.tile([P, P], f32, tag="small", space="PSUM")
            nc.tensor.matmul(w2[:], lhsT=wv_sb[:, 0, 0:P], rhs=wv_sb[:, 0, 0:P],
                             start=True, stop=True)
            w3 = psum.tile([P, P], f32, tag="small", space="PSUM")
            nc.tensor.matmul(w3[:], lhsT=wo_sb[:, 0, 0:P], rhs=wo_sb[:, 0, 0:P],
                             start=True, stop=True)
            w4 = psum.tile([D, D], f32, tag="small", space="PSUM")
            nc.tensor.matmul(w4[:], lhsT=ones96_sb[:], rhs=ones96_sb[:],
                             start=True, stop=True)
            w5 = psum.tile([T, T], f32, tag="small", space="PSUM")
            nc.tensor.matmul(w5[:], lhsT=widc_sb[:], rhs=widc_sb[:],
                             start=True, stop=True)

            for g in range(G):
                # ---- gather + cast + transpose: xT group [128, 3, 512] ----
                xTg = pool.tile([P, 3, 4 * P], bf16, tag="xTg")
                Xfs = []
                for tt in range(4):
                    t = g * 4 + tt
                    Xf = pool.tile([P, C], f32, tag="Xf", bufs=6)
                    nc.gpsimd.indirect_dma_start(
                        out=Xf[:],
                        out_offset=None,
                        in_=x[:],
                        in_offset=IndirectOffsetOnAxis(ap=srci_sb[:, t:t + 1], axis=0),
                    )
                    Xfs.append(Xf)
                    Xb = pool.tile([P, C], bf16, tag="Xb")
                    nc.vector.tensor_copy(Xb[:], Xf[:])
                    pxT = psum.tile([P, 3 * P], bf16, tag="xT", space="PSUM")
                    for cc in range(3):
                        nc.tensor.transpose(
                            pxT[:, cc * P:(cc + 1) * P],
                            Xb[:, cc * P:(cc + 1) * P],
                            identb_sb[:],
                        )
                    dst = xTg[:, :, tt * P:(tt + 1) * P]
                    src = pxT[:].rearrange("p (c t) -> p c t", c=3)
                    nc.vector.tensor_copy(dst, src)

                # ---- q/k projection: channel-major [128, 8, 512] ----
                qkT = pool.tile([P, 8, 4 * P], bf16, tag="qkT")
                for ch in range(8):
                    pqk = psum.tile([P, 4 * P], f32, tag="qk", space="PSUM", bufs=2)
                    for ks in range(3):
                        nc.tensor.matmul(
                            pqk[:],
                            lhsT=wqk_sb[:, ks, ch * P:(ch + 1) * P],
                            rhs=xTg[:, ks, :],
                            start=(ks == 0),
                            stop=(ks == 2),
                        )
                    nc.scalar.copy(qkT[:, ch, :], pqk[:])

                # ---- per-tile: v projection + attention + out ----
                for tt in range(4):
                    t = g * 4 + tt
                    tok = slice(tt * P, (tt + 1) * P)

                    # v token-major [128, 384]
                    pv = psum.tile([P, C], f32, tag="v", space="PSUM")
                    for ks in range(3):
                        nc.tensor.matmul(
                            pv[:],
                            lhsT=xTg[:, ks, tok],
                            rhs=wv_sb[:, ks, :],
                            start=(ks == 0),
                            stop=(ks == 2),
                        )
                    v_sb = pool.tile([P, C], bf16, tag="v_sb", bufs=3)
                    nc.scalar.copy(v_sb[:], pv[:])

                    # window-id row via PE transpose of the id column
                    pwid = psum.tile([P, P], f32, tag="small", space="PSUM")
                    nc.tensor.transpose(
                        pwid[:],
                        widc_sb[:, t:t + 1].to_broadcast([P, P]),
                        identf_sb[:],
                    )
                    eq = pool.tile([P, P], bf16, tag="eq")
                    nc.vector.tensor_tensor(
                        eq[:],
                        widc_sb[:, t:t + 1].to_broadcast([P, P]),
                        pwid[:],
                        op=mybir.AluOpType.is_equal,
                    )

                    # scoresT[j, i] per head, laid out [128, (h i)]
                    ps = psum.tile([P, H * P], f32, tag="score", space="PSUM", bufs=2)
                    for h in range(H):
                        nc.tensor.matmul(
                            ps[:, h * P:(h + 1) * P],
                            lhsT=qkT[:, 4 + h, tok],
                            rhs=qkT[:, h, tok],
                            start=True,
                            stop=True,
                        )
                    probs = pool.tile([P, H * P], bf16, tag="probs", bufs=3)
                    nc.scalar.activation(probs[:], ps[:], ExpF)
                    nc.vector.tensor_tensor(
                        probs[:].rearrange("p (h j) -> p h j", h=H),
                        probs[:].rearrange("p (h j) -> p h j", h=H),
                        eq[:, None, :].to_broadcast([P, H, P]),
                        op=mybir.AluOpType.mult,
                    )

                    # denominator, pre-broadcast across 96 partitions:
                    # denB = ones[128,96]^T @ probsT -> [96, (h i)]
                    pden = psum.tile([D, H * P], f32, tag="small", space="PSUM")
                    nc.tensor.matmul(pden[:], lhsT=ones96_sb[:], rhs=probs[:],
                                     start=True, stop=True)
                    rden = pool.tile([D, H * P], f32, tag="rden")
                    nc.vector.reciprocal(rden[:], pden[:])

                    # out_hT = v_h^T @ probsT -> [96, (h i)]; normalize on evac
                    po = psum.tile([P, H * P], f32, tag="oy", space="PSUM")
                    for h in range(H):
                        nc.tensor.matmul(
                            po[0:D, h * P:(h + 1) * P],
                            lhsT=v_sb[:, h * D:(h + 1) * D],
                            rhs=probs[:, h * P:(h + 1) * P],
                            start=True,
                            stop=True,
                        )
                    hT = pool.tile([D, H, P], bf16, tag="hT")
                    nc.vector.tensor_tensor(
                        hT[:].rearrange("p h j -> p (h j)"),
                        po[0:D, :],
                        rden[:],
                        op=mybir.AluOpType.mult,
                    )

                    # out projection + residual
                    py = psum.tile([P, C], f32, tag="oy", space="PSUM")
                    for h in range(H):
                        nc.tensor.matmul(
                            py[:],
                            lhsT=hT[:, h, :],
                            rhs=wo_sb[:, h, :],
                            start=(h == 0),
                            stop=(h == 3),
                        )
                    y = pool.tile([P, C], f32, tag="y", bufs=3)
                    nc.vector.tensor_add(y[:], py[:], Xfs[tt][:])
                    nc.sync.dma_start(y_out[t * P:(t + 1) * P, :], y[:])

    nc.compile()
    return nc


# ----------------------------------------------------------------------------
# public entry point
# ----------------------------------------------------------------------------

def kernel(**inputs):
    global _last_results
    x = np.asarray(inputs["x"], np.float32)
    labels = np.asarray(inputs["cluster_label"]).reshape(-1).astype(np.int64)
    w_qkv = np.asarray(inputs["w_qkv"], np.float32)
    b_qkv = np.asarray(inputs["b_qkv"], np.float32)
    w_out = np.asarray(inputs["w_out"], np.float32)
    b_out = np.asarray(inputs["b_out"], np.float32)

    if np.any(b_qkv):
        raise NotImplementedError("nonzero b_qkv not supported")

    x2d = np.ascontiguousarray(x.reshape(L, C))
    T, per_core = _build_core_data(labels)
    wqk_h, wv_h, wo_h = _prep_weights(w_qkv, w_out)

    nc = build_program(T)

    bf = ml_dtypes.bfloat16
    identb_h = np.eye(P, dtype=np.float32).astype(bf)
    identf_h = np.eye(P, dtype=np.float32)
    ones96_h = np.ones((P, D), np.float32).astype(bf)

    in_maps = []
    for c in range(NCORES):
        in_maps.append(dict(
            x=x2d,
            wqk=wqk_h,
            wv=wv_h,
            wo=wo_h,
            srci=per_core[c]["srci"],
            widc=per_core[c]["widc"],
            identb=identb_h,
            identf=identf_h,
            ones96=ones96_h,
        ))

    res = run_bass_kernel_spmd(nc, in_maps, core_ids=list(range(NCORES)))
    _last_results = res

    out_sorted = np.empty((L, C), np.float32)
    for c in range(NCORES):
        stage = res.results[c]["y"]
        rows = stage[per_core[c]["valid"]]
        out_sorted[per_core[c]["gpos"]] = rows
    if np.any(b_out):
        out_sorted += b_out[None, :]
    return out_sorted.reshape(B, L, C)


# revision 10
# speedup vs baseline: 1.5079x; 1.5079x over previous
"""Cluster-local attention kernel for Trainium2 (8 NeuronCores, SPMD).

Strategy
--------
Host side (numpy, cheap): replicate the reference's static window layout
(argsort by cluster label, bincount, big-cluster splitting), pack the ragged
windows into 128-slot tiles (best-fit decreasing), and split tiles evenly
across the 8 cores.  Attention is strictly window-local, so each tile is an
independent block-diagonal attention problem described by per-slot arrays:
source row in x (gather index), window id within the tile (mask), and
validity (for host-side compaction of the output).

Device side (Bass/Tile, uniform SPMD program, per-core data):
  per 4-tile group (512 token slots):
    - indirect-DMA gather of x rows -> X [128,384] f32 (kept for residual)
    - cast to bf16, PE-transpose -> xT [128, 3, 512] channel-major
    - QKV projection (bf16 matmuls, fp32 PSUM):
        q,k channel-major [128ch, 8 chunks, 512tok] (heads padded 96->128,
        1/sqrt(D) folded into wq host-side); v token-major [128tok, 384]
    - per tile: scoresT = k_h q_h^T (transposed scores, j on partitions),
      exp on ScalarE (no max subtraction needed -- scores are O(1)),
      block-diagonal mask multiply (window-id equality; the id row is
      materialized by a PE transpose of the id column), denominator via an
      all-ones [128,96] matmul (materializes the den row broadcast across
      96 PSUM partitions for free), reciprocal on DVE, out_hT = v_h^T
      probsT with the 1/den multiply fused into the PSUM->SBUF copy,
      out-projection, residual add, contiguous store to a staging buffer.
Host compacts the staging buffers (drops pad slots) and scatters rows to
their cluster-sorted positions (the reference returns cluster-sorted order).

The Pool engine is used ONLY to issue indirect-gather DMAs -- this image
ships no GpSimd HIPI ucode, so no Pool compute instructions are used.
"""

import os
import sys

sys.path.insert(0, "/opt/trn_rl_repo")

import numpy as np
import ml_dtypes

import concourse.bass as bass
import concourse.tile as tile
from concourse import bacc
from concourse import mybir
from concourse.bass import IndirectOffsetOnAxis
from concourse.bass_utils import run_bass_kernel_spmd

# problem constants (hardcoded per harness contract)
B, L, C, H, NCLUST, CS = 1, 32768, 384, 4, 512, 64
D = C // H  # 96
P = 128
NCORES = 8

f32 = mybir.dt.float32
bf16 = mybir.dt.bfloat16
i32 = mybir.dt.int32

_last_results = None  # stash of BassKernelResults for test.py introspection


# ----------------------------------------------------------------------------
# host-side window layout (replicates reference._windows)
# ----------------------------------------------------------------------------

def _window_layout(labels):
    """Return (index, starts, sizes) of the ragged windows over sorted order."""
    index = np.argsort(labels, kind="stable")
    sizes = np.bincount(labels).tolist()
    new = []
    for s in sizes:
        if s >= CS * 2:
            n = s // CS
            q, r = divmod(s, n)
            new.extend([q + 1 if i < r else q for i in range(n)])
        elif s > 0:
            new.append(s)
    sizes = np.asarray(new, np.int64)
    starts = np.concatenate([[0], np.cumsum(sizes)[:-1]])
    return index, starts, sizes


def _pack_bins(sizes, cap=P):
    """Best-fit decreasing packing of windows into bins of `cap` slots."""
    order = np.argsort(-sizes, kind="stable")
    rem = []
    bins = []
    for w in order:
        s = int(sizes[w])
        best = -1
        best_rem = cap + 1
        for bi, r in enumerate(rem):
            if s <= r < best_rem:
                best, best_rem = bi, r
        if best < 0:
            bins.append([int(w)])
            rem.append(cap - s)
        else:
            bins[best].append(int(w))
            rem[best] -= s
    return bins


def _build_core_data(labels):
    """Build per-core slot arrays (see module docstring)."""
    index, starts, sizes = _window_layout(labels)
    bins = _pack_bins(sizes)
    core_bins = [bins[c::NCORES] for c in range(NCORES)]
    T = max(len(cb) for cb in core_bins)
    T = ((T + 3) // 4) * 4  # kernel processes 4-tile groups

    per_core = []
    for c in range(NCORES):
        srci = np.zeros((T, P), np.int32)
        wid = np.full((T, P), -1.0, np.float32)
        valid = np.zeros((T, P), bool)
        gpos = []
        for t, b in enumerate(core_bins[c]):
            off = 0
            for k, w in enumerate(b):
                s = int(sizes[w])
                sl = slice(off, off + s)
                srci[t, sl] = index[starts[w]: starts[w] + s]
                wid[t, sl] = float(k)
                valid[t, sl] = True
                gpos.extend(range(int(starts[w]), int(starts[w]) + s))
                off += s
        per_core.append(dict(
            srci=np.ascontiguousarray(srci.T),          # [P, T]
            widc=np.ascontiguousarray(wid.T),           # [P, T]
            gpos=np.asarray(gpos, np.int64),
            valid=valid.reshape(-1),
        ))
    return T, per_core


# ----------------------------------------------------------------------------
# weight reorganization (host side)
# ----------------------------------------------------------------------------

def _prep_weights(w_qkv, w_out):
    """Reorganize weights into the SBUF layouts the kernel consumes (bf16)."""
    wq = w_qkv[:, :C] / np.sqrt(np.float32(D))
    wk = w_qkv[:, C:2 * C]
    wv = w_qkv[:, 2 * C:]

    qk = np.zeros((C, 8, P), np.float32)
    for h in range(H):
        qk[:, h, :D] = wq[:, h * D:(h + 1) * D]
        qk[:, 4 + h, :D] = wk[:, h * D:(h + 1) * D]
    qk = qk.reshape(C, 8 * P)                         # [384, 1024]
    wqk = qk.reshape(3, P, 8 * P).transpose(1, 0, 2)  # [128, 3, 1024]

    wv_r = wv.reshape(3, P, C).transpose(1, 0, 2)     # [128, 3, 384]
    wo_r = w_out.reshape(H, D, C).transpose(1, 0, 2)  # [96, 4, 384]

    bf = ml_dtypes.bfloat16
    return (np.ascontiguousarray(wqk).astype(bf),
            np.ascontiguousarray(wv_r).astype(bf),
            np.ascontiguousarray(wo_r).astype(bf))


# ----------------------------------------------------------------------------
# Bass program
# ----------------------------------------------------------------------------

def build_program(T):
    """Build the uniform SPMD Bass program for T tiles (T % 4 == 0)."""
    assert T % 4 == 0
    G = T // 4
    nc = bacc.Bacc("TRN2", target_bir_lowering=False)

    x = nc.dram_tensor("x", [L, C], f32, kind="ExternalInput")
    wqk = nc.dram_tensor("wqk", [P, 3, 8 * P], bf16, kind="ExternalInput")
    wv = nc.dram_tensor("wv", [P, 3, C], bf16, kind="ExternalInput")
    wo = nc.dram_tensor("wo", [D, H, C], bf16, kind="ExternalInput")
    srci = nc.dram_tensor("srci", [P, T], i32, kind="ExternalInput")
    widc = nc.dram_tensor("widc", [P, T], f32, kind="ExternalInput")
    identb = nc.dram_tensor("identb", [P, P], bf16, kind="ExternalInput")
    identf = nc.dram_tensor("identf", [P, P], f32, kind="ExternalInput")
    ones96 = nc.dram_tensor("ones96", [P, D], bf16, kind="ExternalInput")
    y_out = nc.dram_tensor("y", [T * P, C], f32, kind="ExternalOutput")

    ExpF = mybir.ActivationFunctionType.Exp

    with tile.TileContext(nc) as tc:
        with (
            tc.tile_pool(name="const", bufs=1) as cpool,
            tc.tile_pool(name="sb", bufs=2) as pool,
            tc.tile_pool(name="ps", bufs=1, space="PSUM") as psum,
        ):
            # ---- constants ----
            wqk_sb = cpool.tile([P, 3, 8 * P], bf16)
            nc.sync.dma_start(wqk_sb[:], wqk[:])
            wv_sb = cpool.tile([P, 3, C], bf16)
            nc.sync.dma_start(wv_sb[:], wv[:])
            wo_sb = cpool.tile([D, H, C], bf16)
            nc.sync.dma_start(wo_sb[:], wo[:])
            srci_sb = cpool.tile([P, T], i32)
            nc.sync.dma_start(srci_sb[:], srci[:])
            widc_sb = cpool.tile([P, T], f32)
            nc.sync.dma_start(widc_sb[:], widc[:])
            identb_sb = cpool.tile([P, P], bf16)
            nc.sync.dma_start(identb_sb[:], identb[:])
            identf_sb = cpool.tile([P, P], f32)
            nc.sync.dma_start(identf_sb[:], identf[:])
            ones96_sb = cpool.tile([P, D], bf16)
            nc.sync.dma_start(ones96_sb[:], ones96[:])

            # ---- PE warmups ----
            # TRN2 Matmult carries at most one sync-wait command, so make the
            # PE observe each constant's DMA semaphore via a single-input
            # dummy op before any real matmul consumes it.
            warm = psum.tile([P, P], f32, tag="small", space="PSUM")
            nc.tensor.transpose(warm[:], identf_sb[:], identf_sb[:])
            warmb = psum.tile([P, P], bf16, tag="small", space="PSUM")
            nc.tensor.transpose(warmb[:], identb_sb[:], identb_sb[:])
            w1 = psum.tile([P, P], f32, tag="small", space="PSUM")
            nc.tensor.matmul(w1[:], lhsT=wqk_sb[:, 0, 0:P], rhs=wqk_sb[:, 0, 0:P],
                             start=True, stop=True)
            w2 = psum.tile([P, P], f32, tag="small", space="PSUM")
            nc.tensor.matmul(w2[:], lhsT=wv_sb[:, 0, 0:P], rhs=wv_sb[:, 0, 0:P],
                             start=True, stop=True)
            w3 = psum.tile([P, P], f32, tag="small", space="PSUM")
            nc.tensor.matmul(w3[:], lhsT=wo_sb[:, 0, 0:P], rhs=wo_sb[:, 0, 0:P],
                             start=True, stop=True)
            w4 = psum.tile([D, D], f32, tag="small", space="PSUM")
            nc.tensor.matmul(w4[:], lhsT=ones96_sb[:], rhs=ones96_sb[:],
                             start=True, stop=True)
            w5 = psum.tile([T, T], f32, tag="small", space="PSUM")
            nc.tensor.matmul(w5[:], lhsT=widc_sb[:], rhs=widc_sb[:],
                             start=True, stop=True)

            for g in range(G):
                # ---- gather + cast + transpose: xT group [128, 3, 512] ----
                xTg = pool.tile([P, 3, 4 * P], bf16, tag="xTg")
                Xfs = []
                for tt in range(4):
                    t = g * 4 + tt
                    Xf = pool.tile([P, C], f32, tag="Xf", bufs=6)
                    nc.gpsimd.indirect_dma_start(
                        out=Xf[:],
                        out_offset=None,
                        in_=x[:],
                        in_offset=IndirectOffsetOnAxis(ap=srci_sb[:, t:t + 1], axis=0),
                    )
                    Xfs.append(Xf)
                    Xb = pool.tile([P, C], bf16, tag="Xb")
                    nc.vector.tensor_copy(Xb[:], Xf[:])
                    pxT = psum.tile([P, 3 * P], bf16, tag="xT", space="PSUM")
                    for cc in range(3):
                        nc.tensor.transpose(
                            pxT[:, cc * P:(cc + 1) * P],
                            Xb[:, cc * P:(cc + 1) * P],
                            identb_sb[:],
                        )
                    dst = xTg[:, :, tt * P:(tt + 1) * P]
                    src = pxT[:].rearrange("p (c t) -> p c t", c=3)
                    nc.vector.tensor_copy(dst, src)

                # ---- q/k projection: channel-major [128, 8, 512] ----
                qkT = pool.tile([P, 8, 4 * P], bf16, tag="qkT")
                for ch in range(8):
                    pqk = psum.tile([P, 4 * P], f32, tag="qk", space="PSUM", bufs=2)
                    for ks in range(3):
                        nc.tensor.matmul(
                            pqk[:],
                            lhsT=wqk_sb[:, ks, ch * P:(ch + 1) * P],
                            rhs=xTg[:, ks, :],
                            start=(ks == 0),
                            stop=(ks == 2),
                        )
                    nc.scalar.copy(qkT[:, ch, :], pqk[:])

                # ---- per-tile: v projection + attention + out ----
                for tt in range(4):
                    t = g * 4 + tt
                    tok = slice(tt * P, (tt + 1) * P)

                    # v token-major [128, 384]
                    pv = psum.tile([P, C], f32, tag="v", space="PSUM")
                    for ks in range(3):
                        nc.tensor.matmul(
                            pv[:],
                            lhsT=xTg[:, ks, tok],
                            rhs=wv_sb[:, ks, :],
                            start=(ks == 0),
                            stop=(ks == 2),
                        )
                    v_sb = pool.tile([P, C], bf16, tag="v_sb", bufs=3)
                    nc.scalar.copy(v_sb[:], pv[:])

                    # window-id row via PE transpose of the id column
                    pwid = psum.tile([P, P], f32, tag="small", space="PSUM")
                    nc.tensor.transpose(
                        pwid[:],
                        widc_sb[:, t:t + 1].to_broadcast([P, P]),
                        identf_sb[:],
                    )
                    eq = pool.tile([P, P], bf16, tag="eq")
                    nc.vector.tensor_tensor(
                        eq[:],
                        widc_sb[:, t:t + 1].to_broadcast([P, P]),
                        pwid[:],
                        op=mybir.AluOpType.is_equal,
                    )

                    # scoresT[j, i] per head, laid out [128, (h i)]
                    ps = psum.tile([P, H * P], f32, tag="score", space="PSUM", bufs=2)
                    for h in range(H):
                        nc.tensor.matmul(
                            ps[:, h * P:(h + 1) * P],
                            lhsT=qkT[:, 4 + h, tok],
                            rhs=qkT[:, h, tok],
                            start=True,
                            stop=True,
                        )
                    probs = pool.tile([P, H * P], bf16, tag="probs", bufs=3)
                    nc.scalar.activation(probs[:], ps[:], ExpF)
                    nc.vector.tensor_tensor(
                        probs[:].rearrange("p (h j) -> p h j", h=H),
                        probs[:].rearrange("p (h j) -> p h j", h=H),
                        eq[:, None, :].to_broadcast([P, H, P]),
                        op=mybir.AluOpType.mult,
                    )

                    # denominator, pre-broadcast across 96 partitions:
                    # denB = ones[128,96]^T @ probsT -> [96, (h i)]
                    pden = psum.tile([D, H * P], f32, tag="small", space="PSUM")
                    nc.tensor.matmul(pden[:], lhsT=ones96_sb[:], rhs=probs[:],
                                     start=True, stop=True)
                    rden = pool.tile([D, H * P], f32, tag="rden")
                    nc.vector.reciprocal_approx_fast(out=rden[:], in_=pden[:])

                    # out_hT = v_h^T @ probsT -> [96, (h i)]; normalize on evac
                    po = psum.tile([P, H * P], f32, tag="oy", space="PSUM")
                    for h in range(H):
                        nc.tensor.matmul(
                            po[0:D, h * P:(h + 1) * P],
                            lhsT=v_sb[:, h * D:(h + 1) * D],
                            rhs=probs[:, h * P:(h + 1) * P],
                            start=True,
                            stop=True,
                        )
                    hT = pool.tile([D, H, P], bf16, tag="hT")
                    nc.vector.tensor_tensor(
                        hT[:].rearrange("p h j -> p (h j)"),
                        po[0:D, :],
                        rden[:],
                        op=mybir.AluOpType.mult,
                    )

                    # out projection + residual
                    py = psum.tile([P, C], f32, tag="oy", space="PSUM")
                    for h in range(H):
                        nc.tensor.matmul(
                            py[:],
                            lhsT=hT[:, h, :],
                            rhs=wo_sb[:, h, :],
                            start=(h == 0),
                            stop=(h == 3),
                        )
                    y = pool.tile([P, C], f32, tag="y", bufs=3)
                    nc.vector.tensor_add(y[:], py[:], Xfs[tt][:])
                    nc.sync.dma_start(y_out[t * P:(t + 1) * P, :], y[:])

    nc.compile()
    return nc


# ----------------------------------------------------------------------------
# public entry point
# ----------------------------------------------------------------------------

def kernel(**inputs):
    global _last_results
    x = np.asarray(inputs["x"], np.float32)
    labels = np.asarray(inputs["cluster_label"]).reshape(-1).astype(np.int64)
    w_qkv = np.asarray(inputs["w_qkv"], np.float32)
    b_qkv = np.asarray(inputs["b_qkv"], np.float32)
    w_out = np.asarray(inputs["w_out"], np.float32)
    b_out = np.asarray(inputs["b_out"], np.float32)

    if np.any(b_qkv):
        raise NotImplementedError("nonzero b_qkv not supported")

    x2d = np.ascontiguousarray(x.reshape(L, C))
    T, per_core = _build_core_data(labels)
    wqk_h, wv_h, wo_h = _prep_weights(w_qkv, w_out)

    nc = build_program(T)

    bf = ml_dtypes.bfloat16
    identb_h = np.eye(P, dtype=np.float32).astype(bf)
    identf_h = np.eye(P, dtype=np.float32)
    ones96_h = np.ones((P, D), np.float32).astype(bf)

    in_maps = []
    for c in range(NCORES):
        in_maps.append(dict(
            x=x2d,
            wqk=wqk_h,
            wv=wv_h,
            wo=wo_h,
            srci=per_core[c]["srci"],
            widc=per_core[c]["widc"],
            identb=identb_h,
            identf=identf_h,
            ones96=ones96_h,
        ))

    res = run_bass_kernel_spmd(nc, in_maps, core_ids=list(range(NCORES)))
    _last_results = res

    out_sorted = np.empty((L, C), np.float32)
    for c in range(NCORES):
        stage = res.results[c]["y"]
        rows = stage[per_core[c]["valid"]]
        out_sorted[per_core[c]["gpos"]] = rows
    if np.any(b_out):
        out_sorted += b_out[None, :]
    return out_sorted.reshape(B, L, C)
